# revision 1
# baseline (speedup 1.0000x reference)
"""Trainium2 Bass kernel for the GRU session-decoder (nn_Decoder_12506944766179).

Strategy (8 NeuronCores, SPMD):
  - The hidden dimension (2H=2048) is sharded 8-way across cores for the GRU
    recurrence ("gate sharding"): core c owns 256 hidden dims (in a host-side
    even/odd-permuted order so the later max-pair pooling becomes a contiguous
    tensor max). Each step every core computes its slice of the new hidden
    state and an AllGather rebuilds the full hidden state on every core.
  - The input-gate projection gi = emb[x] @ w_ih.T (+biases) is sharded the
    same way (each core computes its own 768 gate columns for all 8192 rows).
  - The output side (max-pair -> lin2 -> +emb residual -> out_embed) is
    sharded over the vocabulary: each core computes a 1251-wide slice of the
    logits for every (t, b), fused into the recurrence loop so the PE work
    hides inside each step's AllGather window.
  - Host side only does dtype casts, transposes, slicing and the final
    concatenation of vocab slices.
"""

import os
import sys

sys.path.insert(0, "/opt/trn_rl_repo")

import numpy as np

import concourse.bass as bass
import concourse.mybir as mybir
import concourse.tile as tile
from concourse import bacc
from concourse.masks import make_identity

V, E, SH, H, B, T = 10004, 512, 1024, 1024, 64, 128
H2 = 2 * H
NCORES = 8
GS = H2 // NCORES          # 256 hidden dims owned per core
G3 = 3 * GS                # 768 gate columns per core
VS = 1251                  # padded vocab slice (8 * 1251 = 10008 >= 10004)
f32 = mybir.dt.float32
bf16 = mybir.dt.bfloat16
i32 = mybir.dt.int32

# dtype knobs for the three matmul groups (f32 correct baseline; bf16 faster)
GRU_BF16 = os.environ.get("K_GRU_BF16", "0") == "1"
B_BF16 = os.environ.get("K_B_BF16", "0") == "1"
D_BF16 = os.environ.get("K_D_BF16", "0") == "1"


def build(nsteps=T):
    gdt = bf16 if GRU_BF16 else f32
    bdt = bf16 if B_BF16 else f32
    ddt = bf16 if D_BF16 else f32
    nrows = nsteps * B

    nc = bacc.Bacc("TRN2", target_bir_lowering=False, debug=False,
                   num_devices=NCORES)

    emb = nc.declare_dram_parameter("emb", [V, E], f32, isOutput=False)
    idx = nc.declare_dram_parameter("idx", [T * B, 1], i32, isOutput=False)
    sesT_d = nc.declare_dram_parameter("sesT", [SH, B], f32, isOutput=False)
    w1ownT_d = nc.declare_dram_parameter("w1ownT", [SH, GS], f32, isOutput=False)
    b1own_d = nc.declare_dram_parameter("b1own", [1, GS], f32, isOutput=False)
    wihT_d = nc.declare_dram_parameter("wihT", [E, G3], f32, isOutput=False)
    gibias_d = nc.declare_dram_parameter("gibias", [128, G3], f32, isOutput=False)
    whhT_d = nc.declare_dram_parameter("whhT", [H2, G3], gdt, isOutput=False)
    bhn_d = nc.declare_dram_parameter("bhn", [1, G3], f32, isOutput=False)
    w2T_d = nc.declare_dram_parameter("w2T", [H, E], ddt, isOutput=False)
    b2_d = nc.declare_dram_parameter("b2", [1, E], f32, isOutput=False)
    woutT_d = nc.declare_dram_parameter("woutT", [E, VS], ddt, isOutput=False)
    out = nc.declare_dram_parameter("out", [nsteps, B, VS], f32, isOutput=True)

    with tile.TileContext(nc) as tc:
        with (
            tc.tile_pool(name="wts", bufs=1) as wts,
            tc.tile_pool(name="sb", bufs=2) as sb,
            tc.tile_pool(name="sbg", bufs=2) as sbg,
            tc.tile_pool(name="pstr", bufs=2, space="PSUM") as pstr,
            tc.tile_pool(name="dram", bufs=1, space="DRAM") as dram,
        ):
            # ---- resident weights / constants -------------------------------
            whh = wts.tile([128, 16 * G3], gdt, name="whh")
            nc.sync.dma_start(
                whh[:].rearrange("p (c n) -> p c n", c=16),
                whhT_d.rearrange("(c p) n -> p c n", p=128))
            wih = wts.tile([128, 4 * G3], bdt, name="wih")
            if bdt == f32:
                nc.sync.dma_start(
                    wih[:].rearrange("p (c n) -> p c n", c=4),
                    wihT_d.rearrange("(c p) n -> p c n", p=128))
            else:
                wihf = sb.tile([128, 4 * G3], f32, name="wihf")
                nc.sync.dma_start(
                    wihf[:].rearrange("p (c n) -> p c n", c=4),
                    wihT_d.rearrange("(c p) n -> p c n", p=128))
                nc.vector.tensor_copy(wih[:], wihf[:])
            w2t = wts.tile([128, 8 * E], ddt, name="w2t")
            nc.sync.dma_start(
                w2t[:].rearrange("p (c n) -> p c n", c=8),
                w2T_d.rearrange("(c p) n -> p c n", p=128))
            wout = wts.tile([128, 4 * VS], ddt, name="wout")
            nc.sync.dma_start(
                wout[:].rearrange("p (c n) -> p c n", c=4),
                woutT_d.rearrange("(c p) n -> p c n", p=128))
            gibias = wts.tile([128, G3], f32, name="gibias")
            nc.sync.dma_start(gibias[:], gibias_d[:])
            bhn = wts.tile([1, G3], gdt, name="bhn")
            if gdt == f32:
                nc.sync.dma_start(bhn[:], bhn_d[:])
            else:
                bhnf = sb.tile([1, G3], f32, name="bhnf")
                nc.sync.dma_start(bhnf[:], bhn_d[:])
                nc.vector.tensor_copy(bhn[:], bhnf[:])
            b2r = wts.tile([1, E], ddt, name="b2r")
            if ddt == f32:
                nc.sync.dma_start(b2r[:], b2_d[:])
            else:
                b2f = sb.tile([1, E], f32, name="b2f")
                nc.sync.dma_start(b2f[:], b2_d[:])
                nc.vector.tensor_copy(b2r[:], b2f[:])
            b1own = wts.tile([1, GS], f32, name="b1own")
            nc.sync.dma_start(b1own[:], b1own_d[:])

            ident = wts.tile([128, 128], f32, name="ident")
            make_identity(nc, ident[:])
            # transposes/PSUM/collective stay f32 always (bf16 PSUM or bf16
            # collective payloads hung the device); only MM operands go bf16
            id_g = id_b = id_d = ident
            ones_g = wts.tile([1, 64], gdt, name="ones_g")
            nc.gpsimd.memset(ones_g[:], 1.0)
            ones_d = wts.tile([1, 64], ddt, name="ones_d")
            nc.gpsimd.memset(ones_d[:], 1.0)
            ones_f = wts.tile([1, 64], f32, name="ones_f")
            nc.gpsimd.memset(ones_f[:], 1.0)

            gi_d = dram.tile([T * B, G3], f32, name="gi_d")
            xe_d = dram.tile([T * B, E], f32, name="xe_d")

            # ---- helper: publish own hidden slice -> AllGather -> full h ----
            def publish(h_own_any):
                # h_own_any: (64, 256) f32 sbuf tile
                trp = pstr.tile([128, 128], f32, name="trp", tag="tr",
                                padded_shape=[128, 512])
                for k in range(2):
                    nc.tensor.transpose(
                        trp[:, 64 * k:64 * (k + 1)],
                        h_own_any[:, 128 * k:128 * (k + 1)],
                        ident[0:64, 0:64])
                hs_own = sb.tile([128, 128], f32, name="hs_own")
                nc.vector.tensor_copy(hs_own[:], trp[:])
                bounce = dram.tile([GS, B], f32, name="bounce", tag="bounce",
                                   bufs=2)
                nc.sync.dma_start(
                    bounce.rearrange("(c p) b -> p c b", p=128),
                    hs_own[:].rearrange("p (c b) -> p c b", c=2))
                hfd = dram.tile([H2, B], f32, name="hfd", tag="hfd", bufs=2,
                                addr_space="Shared")
                nc.gpsimd.collective_compute(
                    "AllGather", mybir.AluOpType.bypass,
                    replica_groups=[list(range(NCORES))],
                    ins=[bounce.opt()], outs=[hfd.opt()])
                hfull = sb.tile([128, 16 * 64], f32, name="hfull", tag="hfull",
                                bufs=3)
                nc.sync.dma_start(
                    hfull[:].rearrange("p (c b) -> p c b", c=16),
                    hfd.rearrange("(c p) b -> p c b", p=128))
                if gdt == bf16:
                    hb = sb.tile([128, 16 * 64], bf16, name="hb", tag="hb",
                                 bufs=3)
                    nc.vector.tensor_copy(hb[:], hfull[:])
                else:
                    hb = hfull
                return hb, hfull

            # ---- phase A: h0 own slice -------------------------------------
            with tc.tile_pool(name="psb", bufs=2, space="PSUM") as psb:
                sest = sb.tile([128, 8 * B], f32, name="sest", bufs=1)
                nc.sync.dma_start(
                    sest[:].rearrange("p (c b) -> p c b", c=8),
                    sesT_d.rearrange("(c p) b -> p c b", p=128))
                w1o = sb.tile([128, 8 * GS], f32, name="w1o", bufs=1)
                nc.sync.dma_start(
                    w1o[:].rearrange("p (c n) -> p c n", c=8),
                    w1ownT_d.rearrange("(c p) n -> p c n", p=128))
                ps0 = psb.tile([128, G3], f32, name="ps0", tag="giB")
                for k in range(8):
                    nc.tensor.matmul(ps0[0:64, 0:GS],
                                     sest[:, 64 * k:64 * (k + 1)],
                                     w1o[:, GS * k:GS * (k + 1)],
                                     start=(k == 0), stop=False)
                nc.tensor.matmul(ps0[0:64, 0:GS], ones_f[:], b1own[:],
                                 start=False, stop=True)
                h_own = sbg.tile([64, GS], f32, name="h_own", tag="hown",
                                 bufs=2)
                nc.scalar.activation(h_own[:], ps0[0:64, 0:GS],
                                     mybir.ActivationFunctionType.Tanh)
                hb, hfull = publish(h_own)

                # ---- phase B: gi = xe @ w_ihT (+bias), all row chunks -------
                for r in range(nrows // 128):
                    idxB = sbg.tile([128, 1], i32, name="idxB", tag="idxB",
                                    bufs=3)
                    nc.sync.dma_start(idxB[:], idx[128 * r:128 * (r + 1), :])
                    xeB = sbg.tile([128, E], f32, name="xeB", tag="xeB",
                                   bufs=3)
                    nc.gpsimd.indirect_dma_start(
                        out=xeB[:], out_offset=None, in_=emb[:],
                        in_offset=bass.IndirectOffsetOnAxis(ap=idxB[:, :1],
                                                            axis=0))
                    psgi = psb.tile([128, G3], f32, name="psgi", tag="giB")
                    for k in range(4):
                        trB = pstr.tile([128, 128], f32, name="trB", tag="tr",
                                        padded_shape=[128, 512])
                        nc.tensor.transpose(trB[:],
                                            xeB[:, 128 * k:128 * (k + 1)],
                                            ident[:])
                        xeT = sbg.tile([128, 128], bdt, name="xeT", tag="xeT",
                                       bufs=3)
                        nc.vector.tensor_copy(xeT[:], trB[:])
                        nc.tensor.matmul(psgi[:, 0:512], xeT[:],
                                         wih[:, G3 * k:G3 * k + 512],
                                         start=(k == 0), stop=(k == 3))
                        nc.tensor.matmul(psgi[:, 512:G3], xeT[:],
                                         wih[:, G3 * k + 512:G3 * (k + 1)],
                                         start=(k == 0), stop=(k == 3))
                    giB = sbg.tile([128, G3], f32, name="giB", tag="giBs",
                                   bufs=3)
                    nc.vector.tensor_add(giB[:], psgi[:], gibias[:])
                    nc.sync.dma_start(gi_d[128 * r:128 * (r + 1), :], giB[:])
                    # stage the gathered embeddings for the output stage's
                    # residual add (avoids a per-step gpsimd indirect gather)
                    nc.sync.dma_start(xe_d[128 * r:128 * (r + 1), :], xeB[:])

            # ---- main loop: GRU step + fused output projection --------------
            with (
                tc.tile_pool(name="psgh", bufs=1, space="PSUM") as psgh,
                tc.tile_pool(name="psmm", bufs=2, space="PSUM") as psmm,
            ):
                def gru_step(t, hb, h_own):
                    gi_t = sb.tile([64, G3], f32, name="gi_t", tag="gi_t",
                                   bufs=3)
                    nc.sync.dma_start(gi_t[:], gi_d[64 * t:64 * (t + 1), :])
                    gh = psgh.tile([64, G3], f32, name="gh", tag="gh")
                    for k in range(16):
                        lhsT = hb[:, 64 * k:64 * (k + 1)]
                        nc.tensor.matmul(gh[:, 0:512], lhsT,
                                         whh[:, G3 * k:G3 * k + 512],
                                         start=(k == 0), stop=(k == 15))
                        nc.tensor.matmul(gh[:, 512:G3], lhsT,
                                         whh[:, G3 * k + 512:G3 * (k + 1)],
                                         start=(k == 0), stop=False)
                    nc.tensor.matmul(gh[:, 512:G3], ones_g[:],
                                     bhn[:, 512:G3], start=False, stop=True)
                    # gates (all batch-major (64, .) tiles)
                    rzp = sbg.tile([64, 512], f32, name="rzp", tag="rzp")
                    nc.vector.tensor_add(rzp[:], gh[:, 0:512], gi_t[:, 0:512])
                    rz = sbg.tile([64, 512], f32, name="rz", tag="rz")
                    nc.scalar.activation(rz[:], rzp[:],
                                         mybir.ActivationFunctionType.Sigmoid)
                    npre = sbg.tile([64, GS], f32, name="npre", tag="npre")
                    nc.vector.tensor_mul(npre[:], rz[:, 0:GS], gh[:, 512:G3])
                    nc.vector.tensor_add(npre[:], npre[:], gi_t[:, 512:G3])
                    ngate = sbg.tile([64, GS], f32, name="ngate", tag="ngate")
                    nc.scalar.activation(ngate[:], npre[:],
                                         mybir.ActivationFunctionType.Tanh)
                    dtile = sbg.tile([64, GS], f32, name="dtile", tag="dtile")
                    nc.vector.tensor_sub(dtile[:], h_own[:], ngate[:])
                    nc.vector.tensor_mul(dtile[:], rz[:, GS:512], dtile[:])
                    h_own2 = sbg.tile([64, GS], f32, name="h_own2", tag="hown",
                                      bufs=2)
                    nc.vector.tensor_add(h_own2[:], ngate[:], dtile[:])
                    hb2, hfull2 = publish(h_own2)
                    return hb2, hfull2, h_own2

                def out_step(t, hfull):
                    # hfull is the f32 state AFTER step t (permuted order):
                    # dec.T = max(h[0:1024], h[1024:2048]) -> (1024, 64)
                    dec = sb.tile([128, 512], f32, name="dec", tag="dec",
                                  bufs=2)
                    nc.vector.tensor_max(dec[:], hfull[:, 0:512],
                                         hfull[:, 512:1024])
                    if ddt != f32:
                        decc = sb.tile([128, 512], ddt, name="decc",
                                       tag="decc", bufs=2)
                        nc.vector.tensor_copy(decc[:], dec[:])
                    else:
                        decc = dec
                    ps2 = psmm.tile([64, E], f32, name="ps2", tag="lin2")
                    for k in range(8):
                        nc.tensor.matmul(ps2[:], decc[:, 64 * k:64 * (k + 1)],
                                         w2t[:, E * k:E * (k + 1)],
                                         start=(k == 0), stop=False)
                    nc.tensor.matmul(ps2[:], ones_d[:], b2r[:],
                                     start=False, stop=True)
                    xe_t = sb.tile([64, E], f32, name="xe_t", tag="xe_t",
                                   bufs=2)
                    nc.sync.dma_start(xe_t[:], xe_d[64 * t:64 * (t + 1), :])
                    dec2 = sb.tile([64, E], f32, name="dec2", tag="dec2",
                                   bufs=2)
                    nc.vector.tensor_add(dec2[:], ps2[:], xe_t[:])
                    trD = pstr.tile([128, 256], f32, name="trD", tag="tr",
                                    padded_shape=[128, 512])
                    for k in range(4):
                        nc.tensor.transpose(trD[:, 64 * k:64 * (k + 1)],
                                            dec2[:, 128 * k:128 * (k + 1)],
                                            ident[0:64, 0:64])
                    d2t = sb.tile([128, 256], ddt, name="d2t", tag="d2t",
                                  bufs=2)
                    nc.vector.tensor_copy(d2t[:], trD[:])
                    lgs = sb.tile([64, VS], f32, name="lgs", tag="lgs",
                                  bufs=2)
                    for n0, n1 in ((0, 512), (512, 1024), (1024, VS)):
                        psl = psmm.tile([64, 512], f32, name="psl", tag="lg")
                        for k in range(4):
                            nc.tensor.matmul(
                                psl[:, 0:n1 - n0],
                                d2t[:, 64 * k:64 * (k + 1)],
                                wout[:, VS * k + n0:VS * k + n1],
                                start=(k == 0), stop=(k == 3))
                        nc.vector.tensor_copy(lgs[:, n0:n1], psl[:, 0:n1 - n0])
                    nc.sync.dma_start(out[t], lgs[:])

                prev_hb, prev_hfull = hb, hfull
                for t in range(nsteps):
                    new_hb, new_hfull, h_own = gru_step(t, prev_hb, h_own)
                    # emitted after step t's PE work: fills the AllGather gap
                    out_step(t - 1, prev_hfull) if t > 0 else None
                    prev_hb, prev_hfull = new_hb, new_hfull
                out_step(nsteps - 1, prev_hfull)

    nc.compile()
    return nc


# ---------------------------------------------------------------------------
# host side
# ---------------------------------------------------------------------------

def _prep_inputs(ses_encoding, x, x_lens, emb_table, w1, b1, w_ih, w_hh,
                 b_ih, b_hh, w2, b2, w_out):
    gdt = np.dtype("float32")
    try:
        import ml_dtypes
        bfnp = np.dtype(ml_dtypes.bfloat16)
    except ImportError:
        bfnp = None

    def cast(a, want_bf):
        a = np.ascontiguousarray(a, dtype=np.float32)
        if want_bf:
            return np.ascontiguousarray(a.astype(bfnp))
        return a

    ses = np.asarray(ses_encoding, np.float32)
    emb = np.ascontiguousarray(np.asarray(emb_table, np.float32))
    w1 = np.asarray(w1, np.float32); b1 = np.asarray(b1, np.float32)
    w_ih = np.asarray(w_ih, np.float32); w_hh = np.asarray(w_hh, np.float32)
    b_ih = np.asarray(b_ih, np.float32); b_hh = np.asarray(b_hh, np.float32)
    w2 = np.asarray(w2, np.float32); b2 = np.asarray(b2, np.float32)
    w_out = np.asarray(w_out, np.float32)

    perm = np.concatenate([np.arange(0, H2, 2), np.arange(1, H2, 2)])
    idxs = np.ascontiguousarray(
        np.asarray(x).astype(np.int32).T.reshape(T * B, 1))
    sesT = np.ascontiguousarray(ses[0].T)                      # (1024, 64)
    w2T = cast(w2.T, D_BF16)                                   # (1024, 512)
    b2r = b2.reshape(1, E).astype(np.float32)
    b_comb = b_ih + np.concatenate([b_hh[0:H2], b_hh[H2:2 * H2],
                                    np.zeros(H2, np.float32)])
    woutT_full = w_out.T                                       # (512, 10004)

    in_maps = []
    for c in range(NCORES):
        od = perm[c * GS:(c + 1) * GS]                         # owned orig dims
        rows = np.concatenate([g * H2 + od for g in range(3)])  # (768,)
        whhT = cast(w_hh[rows][:, perm].T, GRU_BF16)           # (2048, 768)
        wihT = cast(w_ih[rows].T, B_BF16)                      # (512, 768)
        gib = np.tile(b_comb[rows].reshape(1, G3),
                      (128, 1)).astype(np.float32)
        bhn = np.concatenate([np.zeros(512, np.float32),
                              b_hh[2 * H2 + od]]).reshape(1, G3)
        w1ownT = np.ascontiguousarray(w1[od % H].T)            # (1024, 256)
        b1own = b1[od % H].reshape(1, GS).astype(np.float32)
        wsl = woutT_full[:, c * VS:(c + 1) * VS]
        if wsl.shape[1] < VS:
            wsl = np.concatenate(
                [wsl, np.zeros((E, VS - wsl.shape[1]), np.float32)], axis=1)
        woutT = cast(wsl, D_BF16)
        in_maps.append(dict(
            emb=emb, idx=idxs, sesT=sesT, w1ownT=w1ownT, b1own=b1own,
            wihT=wihT, gibias=gib, whhT=whhT, bhn=bhn.astype(np.float32),
            w2T=w2T, b2=b2r, woutT=woutT))
    return in_maps


_CACHED = {}


def _get_runner(nsteps=T):
    key = nsteps
    if key not in _CACHED:
        from concourse.bass_utils import run_bass_kernel_spmd  # noqa
        nc = build(nsteps)
        _CACHED[key] = _SpmdRunner(nc, NCORES)
    return _CACHED[key]


class _SpmdRunner:
    def __init__(self, nc, n_cores):
        import jax
        import jax.numpy as jnp
        from jax.sharding import Mesh, PartitionSpec
        from jax.experimental.shard_map import shard_map
        from concourse.bass2jax import (_bass_exec_p, partition_id_tensor,
                                        install_neuronx_cc_hook)
        self.jax = jax
        self.jnp = jnp
        install_neuronx_cc_hook()
        self.nc = nc
        self.n_cores = n_cores
        in_names, out_names, out_avals = [], [], []
        pname = nc.partition_id_tensor.name if nc.partition_id_tensor else None
        for alloc in nc.m.functions[0].allocations:
            if not isinstance(alloc, mybir.MemoryLocationSet):
                continue
            name = alloc.memorylocations[0].name
            if alloc.kind == "ExternalInput":
                if name != pname:
                    in_names.append(name)
            elif alloc.kind == "ExternalOutput":
                out_names.append(name)
                out_avals.append(jax.core.ShapedArray(
                    tuple(alloc.tensor_shape), mybir.dt.np(alloc.dtype)))
        self.in_names, self.out_names, self.out_avals = \
            in_names, out_names, out_avals
        n_params, n_outs = len(in_names), len(out_avals)
        all_in = in_names + out_names + ([pname] if pname else [])

        def _body(*args):
            operands = list(args)
            if pname is not None:
                operands.append(partition_id_tensor())
            return tuple(_bass_exec_p.bind(
                *operands, out_avals=tuple(out_avals), in_names=tuple(all_in),
                out_names=tuple(out_names), lowering_input_output_aliases=(),
                sim_require_finite=True, sim_require_nnan=True, nc=nc))

        devices = jax.devices()[:n_cores]
        mesh = Mesh(np.asarray(devices), ("core",))
        self.donate = tuple(range(n_params, n_params + n_outs))
        self.sharded = jax.jit(
            shard_map(_body, mesh=mesh,
                      in_specs=(PartitionSpec("core"),) * (n_params + n_outs),
                      out_specs=(PartitionSpec("core"),) * n_outs,
                      check_rep=False),
            donate_argnums=self.donate, keep_unused=True)

    def set_inputs(self, in_maps):
        jax = self.jax
        per_core = [[np.ascontiguousarray(m[n]) for n in self.in_names]
                    for m in in_maps]
        concat = [np.concatenate([per_core[c][i] for c in range(self.n_cores)],
                                 axis=0) for i in range(len(self.in_names))]
        self._dev_in = [jax.device_put(a) for a in concat]
        for a in self._dev_in:
            a.block_until_ready()

    def _zeros(self):
        return [self.jnp.zeros((self.n_cores * av.shape[0], *av.shape[1:]),
                               av.dtype) for av in self.out_avals]

    def run_raw(self):
        outs = self.sharded(*self._dev_in, *self._zeros())
        for o in outs:
            o.block_until_ready()
        return outs

    def results(self):
        outs = self.run_raw()
        res = []
        for c in range(self.n_cores):
            res.append({n: np.asarray(outs[i]).reshape(
                self.n_cores, *self.out_avals[i].shape)[c]
                for i, n in enumerate(self.out_names)})
        return res

    def time(self, iters=10, warmup=2):
        import time as _t
        for _ in range(warmup):
            self.run_raw()
        ts = []
        for _ in range(iters):
            z = self._zeros()
            for zz in z:
                zz.block_until_ready()
            t0 = _t.perf_counter()
            outs = self.sharded(*self._dev_in, *z)
            for o in outs:
                o.block_until_ready()
            ts.append(_t.perf_counter() - t0)
        return min(ts), ts


def kernel(**inputs):
    nsteps = T
    runner = _get_runner(nsteps)
    in_maps = _prep_inputs(**inputs)
    runner.set_inputs(in_maps)
    res = runner.results()
    parts = [res[c]["out"] for c in range(NCORES)]     # each (T, B, VS)
    full = np.concatenate(parts, axis=2)[:, :, :V]     # (T, B, V)
    return np.ascontiguousarray(full.transpose(1, 0, 2))



# revision 2
# speedup vs baseline: 17.2943x; 17.2943x over previous
"""Trainium2 Bass kernel for the GRU session-decoder (nn_Decoder_12506944766179).

Strategy (8 NeuronCores, SPMD):
  - The hidden dimension (2H=2048) is sharded 8-way across cores for the GRU
    recurrence ("gate sharding"): core c owns 256 hidden dims (in a host-side
    even/odd-permuted order so the later max-pair pooling becomes a contiguous
    tensor max). Each step every core computes its slice of the new hidden
    state and an AllGather rebuilds the full hidden state on every core.
  - The input-gate projection gi = emb[x] @ w_ih.T (+biases) is sharded the
    same way (each core computes its own 768 gate columns for all 8192 rows).
  - The output side (max-pair -> lin2 -> +emb residual -> out_embed) is
    sharded over the vocabulary: each core computes a 1251-wide slice of the
    logits for every (t, b), fused into the recurrence loop so the PE work
    hides inside each step's AllGather window.
  - All matmul operands use float32r (same fp32 bytes, 4x faster PE rate than
    float32 when the moving dim is >= 256; the vocab slice is padded to 1280
    so every matmul's moving dim stays >= 256).
  - Inputs and the donated output buffers are placed pre-sharded on the 8
    devices so no cross-device scatter happens inside the timed dispatch.
"""

import sys

sys.path.insert(0, "/opt/trn_rl_repo")

import numpy as np

import concourse.bass as bass
import concourse.mybir as mybir
import concourse.tile as tile
from concourse import bacc
from concourse.masks import make_identity

V, E, SH, H, B, T = 10004, 512, 1024, 1024, 64, 128
H2 = 2 * H
NCORES = 8
GS = H2 // NCORES          # 256 hidden dims owned per core
G3 = 3 * GS                # 768 gate columns per core
VS = 1251                  # vocab slice written out (8 * 1251 = 10008 >= 10004)
VSP = 1280                 # padded vocab slice used on-device (moving dim >= 256)
f32 = mybir.dt.float32
f32r = mybir.dt.float32r
i32 = mybir.dt.int32


def build(nsteps=T):
    mdt = f32r
    nrows = nsteps * B

    nc = bacc.Bacc("TRN2", target_bir_lowering=False, debug=False,
                   num_devices=NCORES)

    emb = nc.declare_dram_parameter("emb", [V, E], f32, isOutput=False)
    idx = nc.declare_dram_parameter("idx", [T * B, 1], i32, isOutput=False)
    sesT_d = nc.declare_dram_parameter("sesT", [SH, B], mdt, isOutput=False)
    w1ownT_d = nc.declare_dram_parameter("w1ownT", [SH, GS], mdt, isOutput=False)
    b1own_d = nc.declare_dram_parameter("b1own", [1, GS], mdt, isOutput=False)
    wihT_d = nc.declare_dram_parameter("wihT", [E, G3], mdt, isOutput=False)
    gibias_d = nc.declare_dram_parameter("gibias", [128, G3], f32, isOutput=False)
    whhT_d = nc.declare_dram_parameter("whhT", [H2, G3], mdt, isOutput=False)
    bhn_d = nc.declare_dram_parameter("bhn", [1, G3], mdt, isOutput=False)
    w2T_d = nc.declare_dram_parameter("w2T", [H, E], mdt, isOutput=False)
    b2_d = nc.declare_dram_parameter("b2", [1, E], mdt, isOutput=False)
    woutT_d = nc.declare_dram_parameter("woutT", [E, VSP], mdt, isOutput=False)
    ones_d = nc.declare_dram_parameter("ones", [1, 64], mdt, isOutput=False)
    out = nc.declare_dram_parameter("out", [nsteps, B, VS], f32, isOutput=True)

    with tile.TileContext(nc) as tc:
        with (
            tc.tile_pool(name="wts", bufs=1) as wts,
            tc.tile_pool(name="sb", bufs=2) as sb,
            tc.tile_pool(name="sbg", bufs=2) as sbg,
            tc.tile_pool(name="pstr", bufs=2, space="PSUM") as pstr,
            tc.tile_pool(name="dram", bufs=1, space="DRAM") as dram,
        ):
            # ---- resident weights / constants -------------------------------
            whh = wts.tile([128, 16 * G3], mdt, name="whh")
            nc.sync.dma_start(
                whh[:].rearrange("p (c n) -> p c n", c=16),
                whhT_d.rearrange("(c p) n -> p c n", p=128))
            wih = wts.tile([128, 4 * G3], mdt, name="wih")
            nc.sync.dma_start(
                wih[:].rearrange("p (c n) -> p c n", c=4),
                wihT_d.rearrange("(c p) n -> p c n", p=128))
            w2t = wts.tile([128, 8 * E], mdt, name="w2t")
            nc.sync.dma_start(
                w2t[:].rearrange("p (c n) -> p c n", c=8),
                w2T_d.rearrange("(c p) n -> p c n", p=128))
            wout = wts.tile([128, 4 * VSP], mdt, name="wout")
            nc.sync.dma_start(
                wout[:].rearrange("p (c n) -> p c n", c=4),
                woutT_d.rearrange("(c p) n -> p c n", p=128))
            gibias = wts.tile([128, G3], f32, name="gibias")
            nc.sync.dma_start(gibias[:], gibias_d[:])
            bhn = wts.tile([1, G3], mdt, name="bhn")
            nc.sync.dma_start(bhn[:], bhn_d[:])
            b2r = wts.tile([1, E], mdt, name="b2r")
            nc.sync.dma_start(b2r[:], b2_d[:])
            b1own = wts.tile([1, GS], mdt, name="b1own")
            nc.sync.dma_start(b1own[:], b1own_d[:])

            ident = wts.tile([128, 128], f32, name="ident")
            make_identity(nc, ident[:])
            ones_g = wts.tile([1, 64], mdt, name="ones_g")
            nc.sync.dma_start(ones_g[:], ones_d[:])

            gi_d = dram.tile([T * B, G3], f32, name="gi_d")
            xe_d = dram.tile([T * B, E], f32, name="xe_d")

            # ---- helper: publish own hidden slice -> AllGather -> full h ----
            def publish(h_own_any):
                trp = pstr.tile([128, 128], f32, name="trp", tag="tr",
                                padded_shape=[128, 512])
                for k in range(2):
                    nc.tensor.transpose(
                        trp[:, 64 * k:64 * (k + 1)],
                        h_own_any[:, 128 * k:128 * (k + 1)],
                        ident[0:64, 0:64])
                hs_own = sb.tile([128, 128], mdt, name="hs_own")
                nc.vector.tensor_copy(hs_own[:], trp[:])
                bounce = dram.tile([GS, B], mdt, name="bounce", tag="bounce",
                                   bufs=2)
                nc.sync.dma_start(
                    bounce.rearrange("(c p) b -> p c b", p=128),
                    hs_own[:].rearrange("p (c b) -> p c b", c=2))
                hfd = dram.tile([H2, B], mdt, name="hfd", tag="hfd", bufs=2,
                                addr_space="Shared")
                nc.gpsimd.collective_compute(
                    "AllGather", mybir.AluOpType.bypass,
                    replica_groups=[list(range(NCORES))],
                    ins=[bounce.opt()], outs=[hfd.opt()])
                hfull = sb.tile([128, 16 * 64], mdt, name="hfull", tag="hfull",
                                bufs=3)
                nc.sync.dma_start(
                    hfull[:].rearrange("p (c b) -> p c b", c=16),
                    hfd.rearrange("(c p) b -> p c b", p=128))
                return hfull

            # ---- phase A: h0 own slice -------------------------------------
            with tc.tile_pool(name="psb", bufs=2, space="PSUM") as psb:
                sest = sb.tile([128, 8 * B], mdt, name="sest", bufs=1)
                nc.sync.dma_start(
                    sest[:].rearrange("p (c b) -> p c b", c=8),
                    sesT_d.rearrange("(c p) b -> p c b", p=128))
                w1o = sb.tile([128, 8 * GS], mdt, name="w1o", bufs=1)
                nc.sync.dma_start(
                    w1o[:].rearrange("p (c n) -> p c n", c=8),
                    w1ownT_d.rearrange("(c p) n -> p c n", p=128))
                ps0 = psb.tile([128, G3], f32, name="ps0", tag="giB")
                for k in range(8):
                    nc.tensor.matmul(ps0[0:64, 0:GS],
                                     sest[:, 64 * k:64 * (k + 1)],
                                     w1o[:, GS * k:GS * (k + 1)],
                                     start=(k == 0), stop=False)
                nc.tensor.matmul(ps0[0:64, 0:GS], ones_g[:], b1own[:],
                                 start=False, stop=True)
                h_own = sbg.tile([64, GS], f32, name="h_own", tag="hown",
                                 bufs=2)
                nc.scalar.activation(h_own[:], ps0[0:64, 0:GS],
                                     mybir.ActivationFunctionType.Tanh)
                hfull = publish(h_own)

                # ---- phase B: gi = xe @ w_ihT (+bias), all row chunks -------
                for r in range(nrows // 128):
                    idxB = sbg.tile([128, 1], i32, name="idxB", tag="idxB",
                                    bufs=3)
                    nc.sync.dma_start(idxB[:], idx[128 * r:128 * (r + 1), :])
                    xeB = sbg.tile([128, E], f32, name="xeB", tag="xeB",
                                   bufs=3)
                    nc.gpsimd.indirect_dma_start(
                        out=xeB[:], out_offset=None, in_=emb[:],
                        in_offset=bass.IndirectOffsetOnAxis(ap=idxB[:, :1],
                                                            axis=0))
                    psgi = psb.tile([128, G3], f32, name="psgi", tag="giB")
                    for k in range(4):
                        trB = pstr.tile([128, 128], f32, name="trB", tag="tr",
                                        padded_shape=[128, 512])
                        nc.tensor.transpose(trB[:],
                                            xeB[:, 128 * k:128 * (k + 1)],
                                            ident[:])
                        xeT = sbg.tile([128, 128], mdt, name="xeT", tag="xeT",
                                       bufs=3)
                        nc.vector.tensor_copy(xeT[:], trB[:])
                        nc.tensor.matmul(psgi[:, 0:512], xeT[:],
                                         wih[:, G3 * k:G3 * k + 512],
                                         start=(k == 0), stop=(k == 3))
                        nc.tensor.matmul(psgi[:, 512:G3], xeT[:],
                                         wih[:, G3 * k + 512:G3 * (k + 1)],
                                         start=(k == 0), stop=(k == 3))
                    giB = sbg.tile([128, G3], f32, name="giB", tag="giBs",
                                   bufs=3)
                    nc.vector.tensor_add(giB[:], psgi[:], gibias[:])
                    nc.sync.dma_start(gi_d[128 * r:128 * (r + 1), :], giB[:])
                    # stage the gathered embeddings for the output stage's
                    # residual add (avoids a per-step gpsimd indirect gather)
                    nc.sync.dma_start(xe_d[128 * r:128 * (r + 1), :], xeB[:])

            # ---- main loop: GRU step + fused output projection --------------
            with (
                tc.tile_pool(name="psgh", bufs=1, space="PSUM") as psgh,
                tc.tile_pool(name="psmm", bufs=2, space="PSUM") as psmm,
            ):
                def gru_step(t, hb, h_own):
                    gi_t = sb.tile([64, G3], f32, name="gi_t", tag="gi_t",
                                   bufs=3)
                    nc.sync.dma_start(gi_t[:], gi_d[64 * t:64 * (t + 1), :])
                    gh = psgh.tile([64, G3], f32, name="gh", tag="gh")
                    for k in range(16):
                        lhsT = hb[:, 64 * k:64 * (k + 1)]
                        nc.tensor.matmul(gh[:, 0:512], lhsT,
                                         whh[:, G3 * k:G3 * k + 512],
                                         start=(k == 0), stop=(k == 15))
                        nc.tensor.matmul(gh[:, 512:G3], lhsT,
                                         whh[:, G3 * k + 512:G3 * (k + 1)],
                                         start=(k == 0), stop=False)
                    nc.tensor.matmul(gh[:, 512:G3], ones_g[:],
                                     bhn[:, 512:G3], start=False, stop=True)
                    # gates (all batch-major (64, .) tiles)
                    rzp = sbg.tile([64, 512], f32, name="rzp", tag="rzp")
                    nc.vector.tensor_add(rzp[:], gh[:, 0:512], gi_t[:, 0:512])
                    rz = sbg.tile([64, 512], f32, name="rz", tag="rz")
                    nc.scalar.activation(rz[:], rzp[:],
                                         mybir.ActivationFunctionType.Sigmoid)
                    npre = sbg.tile([64, GS], f32, name="npre", tag="npre")
                    nc.vector.tensor_mul(npre[:], rz[:, 0:GS], gh[:, 512:G3])
                    nc.vector.tensor_add(npre[:], npre[:], gi_t[:, 512:G3])
                    ngate = sbg.tile([64, GS], f32, name="ngate", tag="ngate")
                    nc.scalar.activation(ngate[:], npre[:],
                                         mybir.ActivationFunctionType.Tanh)
                    dtile = sbg.tile([64, GS], f32, name="dtile", tag="dtile")
                    nc.vector.tensor_sub(dtile[:], h_own[:], ngate[:])
                    nc.vector.tensor_mul(dtile[:], rz[:, GS:512], dtile[:])
                    h_own2 = sbg.tile([64, GS], f32, name="h_own2", tag="hown",
                                      bufs=2)
                    nc.vector.tensor_add(h_own2[:], ngate[:], dtile[:])
                    hfull2 = publish(h_own2)
                    return hfull2, h_own2

                def out_step(t, hfull):
                    # hfull is the f32r state AFTER step t (permuted order):
                    # dec.T = max(h[0:1024], h[1024:2048]) -> (1024, 64)
                    decc = sb.tile([128, 512], mdt, name="dec", tag="dec",
                                   bufs=2)
                    nc.vector.tensor_max(decc[:], hfull[:, 0:512],
                                         hfull[:, 512:1024])
                    ps2 = psmm.tile([64, E], f32, name="ps2", tag="lin2")
                    for k in range(8):
                        nc.tensor.matmul(ps2[:], decc[:, 64 * k:64 * (k + 1)],
                                         w2t[:, E * k:E * (k + 1)],
                                         start=(k == 0), stop=False)
                    nc.tensor.matmul(ps2[:], ones_g[:], b2r[:],
                                     start=False, stop=True)
                    xe_t = sb.tile([64, E], f32, name="xe_t", tag="xe_t",
                                   bufs=2)
                    nc.sync.dma_start(xe_t[:], xe_d[64 * t:64 * (t + 1), :])
                    dec2 = sb.tile([64, E], f32, name="dec2", tag="dec2",
                                   bufs=2)
                    nc.vector.tensor_add(dec2[:], ps2[:], xe_t[:])
                    trD = pstr.tile([128, 256], f32, name="trD", tag="tr",
                                    padded_shape=[128, 512])
                    for k in range(4):
                        nc.tensor.transpose(trD[:, 64 * k:64 * (k + 1)],
                                            dec2[:, 128 * k:128 * (k + 1)],
                                            ident[0:64, 0:64])
                    d2t = sb.tile([128, 256], mdt, name="d2t", tag="d2t",
                                  bufs=2)
                    nc.vector.tensor_copy(d2t[:], trD[:])
                    lgs = sb.tile([64, VSP], f32, name="lgs", tag="lgs",
                                  bufs=2)
                    for n0, n1 in ((0, 512), (512, 1024), (1024, VSP)):
                        psl = psmm.tile([64, 512], f32, name="psl", tag="lg")
                        for k in range(4):
                            nc.tensor.matmul(
                                psl[:, 0:n1 - n0],
                                d2t[:, 64 * k:64 * (k + 1)],
                                wout[:, VSP * k + n0:VSP * k + n1],
                                start=(k == 0), stop=(k == 3))
                        nc.vector.tensor_copy(lgs[:, n0:n1], psl[:, 0:n1 - n0])
                    nc.sync.dma_start(out[t], lgs[:, 0:VS])

                prev_hfull = hfull
                for t in range(nsteps):
                    new_hfull, h_own = gru_step(t, prev_hfull, h_own)
                    # emitted after step t's PE work: fills the AllGather gap
                    out_step(t - 1, prev_hfull) if t > 0 else None
                    prev_hfull = new_hfull
                out_step(nsteps - 1, prev_hfull)

    nc.compile()
    return nc


# ---------------------------------------------------------------------------
# host side
# ---------------------------------------------------------------------------

def _prep_inputs(ses_encoding, x, x_lens, emb_table, w1, b1, w_ih, w_hh,
                 b_ih, b_hh, w2, b2, w_out):
    ses = np.asarray(ses_encoding, np.float32)
    emb = np.ascontiguousarray(np.asarray(emb_table, np.float32))
    w1 = np.asarray(w1, np.float32); b1 = np.asarray(b1, np.float32)
    w_ih = np.asarray(w_ih, np.float32); w_hh = np.asarray(w_hh, np.float32)
    b_ih = np.asarray(b_ih, np.float32); b_hh = np.asarray(b_hh, np.float32)
    w2 = np.asarray(w2, np.float32); b2 = np.asarray(b2, np.float32)
    w_out = np.asarray(w_out, np.float32)

    perm = np.concatenate([np.arange(0, H2, 2), np.arange(1, H2, 2)])
    idxs = np.ascontiguousarray(
        np.asarray(x).astype(np.int32).T.reshape(T * B, 1))
    sesT = np.ascontiguousarray(ses[0].T)                      # (1024, 64)
    w2T = np.ascontiguousarray(w2.T)                           # (1024, 512)
    b2r = b2.reshape(1, E).astype(np.float32)
    b_comb = b_ih + np.concatenate([b_hh[0:H2], b_hh[H2:2 * H2],
                                    np.zeros(H2, np.float32)])
    woutT_full = w_out.T                                       # (512, 10004)
    ones = np.ones((1, 64), np.float32)

    in_maps = []
    for c in range(NCORES):
        od = perm[c * GS:(c + 1) * GS]                         # owned orig dims
        rows = np.concatenate([g * H2 + od for g in range(3)])  # (768,)
        whhT = np.ascontiguousarray(w_hh[rows][:, perm].T)     # (2048, 768)
        wihT = np.ascontiguousarray(w_ih[rows].T)              # (512, 768)
        gib = np.tile(b_comb[rows].reshape(1, G3),
                      (128, 1)).astype(np.float32)
        bhn = np.concatenate([np.zeros(512, np.float32),
                              b_hh[2 * H2 + od]]).reshape(1, G3)
        w1ownT = np.ascontiguousarray(w1[od % H].T)            # (1024, 256)
        b1own = b1[od % H].reshape(1, GS).astype(np.float32)
        wsl = woutT_full[:, c * VS:(c + 1) * VS]
        wsl = np.concatenate(
            [wsl, np.zeros((E, VSP - wsl.shape[1]), np.float32)], axis=1)
        woutT = np.ascontiguousarray(wsl)
        in_maps.append(dict(
            emb=emb, idx=idxs, sesT=sesT, w1ownT=w1ownT, b1own=b1own,
            wihT=wihT, gibias=gib, whhT=whhT, bhn=bhn.astype(np.float32),
            w2T=w2T, b2=b2r, woutT=woutT, ones=ones))
    return in_maps


_CACHED = {}


def _get_runner(nsteps=T):
    key = nsteps
    if key not in _CACHED:
        nc = build(nsteps)
        _CACHED[key] = _SpmdRunner(nc, NCORES)
    return _CACHED[key]


class _SpmdRunner:
    def __init__(self, nc, n_cores):
        import jax
        import jax.numpy as jnp
        from jax.sharding import Mesh, PartitionSpec, NamedSharding
        from jax.experimental.shard_map import shard_map
        from concourse.bass2jax import (_bass_exec_p, partition_id_tensor,
                                        install_neuronx_cc_hook)
        self.jax = jax
        self.jnp = jnp
        install_neuronx_cc_hook()
        self.nc = nc
        self.n_cores = n_cores
        in_names, out_names, out_avals = [], [], []
        pname = nc.partition_id_tensor.name if nc.partition_id_tensor else None
        for alloc in nc.m.functions[0].allocations:
            if not isinstance(alloc, mybir.MemoryLocationSet):
                continue
            name = alloc.memorylocations[0].name
            if alloc.kind == "ExternalInput":
                if name != pname:
                    in_names.append(name)
            elif alloc.kind == "ExternalOutput":
                out_names.append(name)
                out_avals.append(jax.core.ShapedArray(
                    tuple(alloc.tensor_shape), mybir.dt.np(alloc.dtype)))
        self.in_names, self.out_names, self.out_avals = \
            in_names, out_names, out_avals
        n_params, n_outs = len(in_names), len(out_avals)
        all_in = in_names + out_names + ([pname] if pname else [])

        def _body(*args):
            operands = list(args)
            if pname is not None:
                operands.append(partition_id_tensor())
            return tuple(_bass_exec_p.bind(
                *operands, out_avals=tuple(out_avals), in_names=tuple(all_in),
                out_names=tuple(out_names), lowering_input_output_aliases=(),
                sim_require_finite=True, sim_require_nnan=True, nc=nc))

        devices = jax.devices()[:n_cores]
        self.mesh = mesh = Mesh(np.asarray(devices), ("core",))
        self.sharding = NamedSharding(mesh, PartitionSpec("core"))
        self.donate = tuple(range(n_params, n_params + n_outs))
        self.sharded = jax.jit(
            shard_map(_body, mesh=mesh,
                      in_specs=(PartitionSpec("core"),) * (n_params + n_outs),
                      out_specs=(PartitionSpec("core"),) * n_outs,
                      check_rep=False),
            donate_argnums=self.donate, keep_unused=True)
        # on-device sharded zero maker for the donated output buffers
        self._mk_zeros = jax.jit(
            lambda: tuple(
                jnp.zeros((self.n_cores * av.shape[0], *av.shape[1:]),
                          av.dtype) for av in out_avals),
            out_shardings=tuple(self.sharding for _ in out_avals))

    def set_inputs(self, in_maps):
        jax = self.jax
        per_core = [[np.ascontiguousarray(m[n]) for n in self.in_names]
                    for m in in_maps]
        concat = [np.concatenate([per_core[c][i] for c in range(self.n_cores)],
                                 axis=0) for i in range(len(self.in_names))]
        self._dev_in = [jax.device_put(a, self.sharding) for a in concat]
        for a in self._dev_in:
            a.block_until_ready()

    def _zeros(self):
        return list(self._mk_zeros())

    def run_raw(self):
        outs = self.sharded(*self._dev_in, *self._zeros())
        for o in outs:
            o.block_until_ready()
        return outs

    def results(self):
        outs = self.run_raw()
        res = []
        for c in range(self.n_cores):
            res.append({n: np.asarray(outs[i]).reshape(
                self.n_cores, *self.out_avals[i].shape)[c]
                for i, n in enumerate(self.out_names)})
        return res

    def time(self, iters=10, warmup=2):
        import time as _t
        for _ in range(warmup):
            self.run_raw()
        ts = []
        for _ in range(iters):
            z = self._zeros()
            for zz in z:
                zz.block_until_ready()
            t0 = _t.perf_counter()
            outs = self.sharded(*self._dev_in, *z)
            for o in outs:
                o.block_until_ready()
            ts.append(_t.perf_counter() - t0)
        return min(ts), ts


def kernel(**inputs):
    nsteps = T
    runner = _get_runner(nsteps)
    in_maps = _prep_inputs(**inputs)
    runner.set_inputs(in_maps)
    res = runner.results()
    parts = [res[c]["out"] for c in range(NCORES)]     # each (T, B, VS)
    full = np.concatenate(parts, axis=2)[:, :, :V]     # (T, B, V)
    return np.ascontiguousarray(full.transpose(1, 0, 2))
